# revision 15
# baseline (speedup 1.0000x reference)
"""Trainium2 Bass kernel for nn_MixOp (hard gumbel-softmax routed conv+BN+ReLU).

Forward semantics (from the reference):
  index  = argmax(softmax((logits + g) / TAU))            # routing, 5 branches
  y      = relu(conv(x, w[index]) * inv + (beta - mean*inv))   for that branch
  out    = y * take(onehot + soft - stop_grad(soft), index) == y * 1.0  (exact)

Only the selected branch runs.  Routing is evaluated on host (5 scalars,
mirroring the reference's lax.switch dispatch); the conv+BN+ReLU runs on 8
NeuronCores, data-parallel over batch (4 images per core).

Per-core conv formulation: for each output channel `co` the KxK conv is a sum
over (kw, ci) of 1-D convolutions along H.  Each 1-D H-conv is one matmul on
the PE array:
    stationary lhsT = Toeplitz band T[hi, ho] = w[hi-ho, kw, ci, co]   (128 x HO)
    moving rhs      = x_tile[:, kw : kw+512, ci]                       (128 x 512)
accumulated over the K*cin_g (kw, ci) passes in one PSUM bank.  H is tiled in
bands of HO = 128 - 2*pad output rows; the ragged last rows of all 4 images
are packed block-diagonally into one extra "tail" matmul set.  Zero padding
(SAME) is pre-applied on the host so every SBUF x-tile is written by exactly
one DMA (avoids extra cross-engine sync waits on the matmuls).  BN+ReLU is
fused into the PSUM->SBUF eviction on the scalar engine (relu(scale*x+bias)),
writing channel-strided into an NHWC tile that DMAs out as contiguous rows.
"""

import os
import sys

import numpy as np

for _p in ("/opt/trn_rl_repo",):
    if _p not in sys.path and os.path.isdir(_p):
        sys.path.insert(0, _p)

TAU = 1.0
EPS = 1e-5
GROUPS = (1, 1, 4, 1, 4)
KSIZES = (1, 3, 3, 5, 5)
B, H, W, C = 32, 512, 512, 4
N_CORES = 8
B_SH = B // N_CORES  # images per core

# Matmul precision: "fp32" (exact, 4 cyc/row), "bf16" (1 cyc/row, ~3e-3 rel),
# "f32r" (fp32 bits, reduced-precision multiply, 1 cyc/row at >=256 cols).
MODE = os.environ.get("MIXOP_MODE", "fp32")

# Stash of the last BassKernelResults (exec_time_ns etc.) for the local harness.
LAST_RESULTS = None


def _ensure_ntff_hook():
    """Make `antenv.axon_hooks` importable so run_bass_kernel_spmd(trace=True)
    can NTFF-profile under axon (or degrade gracefully instead of crashing)."""
    import types
    import contextlib
    import ctypes

    try:
        import antenv.axon_hooks  # noqa: F401

        return
    except ImportError:
        pass
    try:
        import antenv
    except ImportError:
        return
    mod = types.ModuleType("antenv.axon_hooks")
    _hook = [None]
    mod.set_axon_ntff_profile_hook = lambda h: _hook.__setitem__(0, h)
    mod.get_axon_ntff_profile_hook = lambda: _hook[0]
    sys.modules["antenv.axon_hooks"] = mod
    antenv.axon_hooks = mod

    so_path = "/opt/axon/libaxon_pjrt.so"
    if not os.path.exists(so_path):
        return
    try:
        lib = ctypes.CDLL(so_path)
        if not hasattr(lib, "axon_start_nrt_profile"):
            return
        lib.axon_start_nrt_profile.argtypes = [
            ctypes.POINTER(ctypes.c_int64),
            ctypes.c_size_t,
        ]
        lib.axon_start_nrt_profile.restype = ctypes.c_int64
        lib.axon_stop_nrt_profile.argtypes = [ctypes.c_char_p]
        lib.axon_stop_nrt_profile.restype = ctypes.c_int64

        @contextlib.contextmanager
        def _ntff_hook(output_dir, device_ids):
            import jax

            jax.devices()
            if device_ids:
                ids = (ctypes.c_int64 * len(device_ids))(*device_ids)
                rc = lib.axon_start_nrt_profile(ids, len(device_ids))
            else:
                rc = lib.axon_start_nrt_profile(None, 0)
            if rc != 0:
                raise RuntimeError(f"axon_start_nrt_profile rc={rc}")
            try:
                yield
            finally:
                n = lib.axon_stop_nrt_profile(str(output_dir).encode())
                print(f"ntff profile: {n} file(s) written to {output_dir}")

        mod.set_axon_ntff_profile_hook(_ntff_hook)
    except Exception:
        pass


def _routing_index(logits, g):
    s = (np.asarray(logits, np.float32) + np.asarray(g, np.float32)) / np.float32(TAU)
    e = np.exp(s - s.max())
    soft = e / e.sum()
    return int(np.argmax(soft))


def _build_toeplitz(w, K, groups, HO, ho_rem, np_dt):
    """Host-built stationary stacks.

    Returns (tfull [128, S, HO], ttail [128, S, 4*ho_rem] | None,
             pairs: per-co list of (kw, ci_moving) in stationary order).
    Stationary s-index order: co-major, then kw, then ci.
    """
    pad = K // 2
    cin_g = C // groups
    S = 4 * K * cin_g
    hin_rem = ho_rem + 2 * pad

    tfull = np.zeros((128, S, HO), np.float32)
    ttail = np.zeros((128, S, 4 * ho_rem), np.float32) if ho_rem else None
    pairs = []
    jo = np.arange(HO)
    jt = np.arange(ho_rem)
    s = 0
    for co in range(4):
        plist = []
        for kw in range(K):
            for ci in range(cin_g):
                ci_mov = co if groups == 4 else ci
                plist.append((kw, ci_mov))
                for kh in range(K):
                    wv = np.float32(w[kh, kw, 0 if groups == 4 else ci, co])
                    tfull[jo + kh, s, jo] = wv
                    if ttail is not None:
                        for i in range(4):
                            ttail[32 * i + jt + kh, s, ho_rem * i + jt] = wv
                s += 1
        pairs.append(plist)
    assert s == S and hin_rem <= 32
    return tfull.astype(np_dt), (None if ttail is None else ttail.astype(np_dt)), pairs


def _build_program(K, pairs, S, HO, ho_rem, inv, bvec, dt_in):
    import concourse.bacc as bacc
    import concourse.mybir as mybir
    import concourse.tile as tile
    from contextlib import ExitStack

    pad = K // 2
    WP = W + 2 * pad  # padded width
    HP = H + 2 * pad  # padded height
    relu = mybir.ActivationFunctionType.Relu

    nc = bacc.Bacc()
    xx = nc.declare_dram_parameter("xpad", [B_SH, HP, WP, C], dt_in, isOutput=False)
    tf = nc.declare_dram_parameter("tfull", [128, S, HO], dt_in, isOutput=False)
    xt_d = tt = None
    if ho_rem:
        xt_d = nc.declare_dram_parameter("xtail", [128, WP, C], dt_in, isOutput=False)
        tt = nc.declare_dram_parameter(
            "ttail", [128, S, 4 * ho_rem], dt_in, isOutput=False
        )
    yy = nc.declare_dram_parameter("y", [B_SH, H, W, C], mybir.dt.float32, isOutput=True)

    with tile.TileContext(nc) as tc, ExitStack() as ctx:
        singles = ctx.enter_context(tc.tile_pool(name="singles", bufs=1))
        xpool = ctx.enter_context(tc.tile_pool(name="xpool", bufs=8))
        ypool = ctx.enter_context(tc.tile_pool(name="ypool", bufs=6))
        tailpool = ctx.enter_context(tc.tile_pool(name="tailpool", bufs=1))
        pspool = ctx.enter_context(tc.tile_pool(name="pspool", bufs=8, space="PSUM"))

        bias_sb = singles.tile([128, 4], mybir.dt.float32)
        for co in range(4):
            nc.vector.memset(bias_sb[:, co : co + 1], float(bvec[co]))

        t_sb = singles.tile([128, S, HO], dt_in)
        nc.sync.dma_start(out=t_sb, in_=tf[:, :, :])
        tt_sb = None
        if ho_rem:
            tt_sb = singles.tile([128, S, 4 * ho_rem], dt_in)
            nc.sync.dma_start(out=tt_sb, in_=tt[:, :, :])

        def do_co(psum_t, x_t, co, n_out, lhs_tile):
            plist = pairs[co]
            n = len(plist)
            for t, (kw, ci) in enumerate(plist):
                nc.tensor.matmul(
                    out=psum_t[0:n_out, 0:512],
                    lhsT=lhs_tile[:, co * n + t, 0:n_out],
                    rhs=x_t[:, kw : kw + W, ci],
                    start=(t == 0),
                    stop=(t == n - 1),
                )

        def evict(psum_t, y_t, co, n_out):
            nc.scalar.activation(
                out=y_t[0:n_out, :, co],
                in_=psum_t[0:n_out, 0:512],
                func=relu,
                scale=float(inv[co]),
                bias=bias_sb[0:n_out, co : co + 1],
            )

        for img in range(B_SH):
            x_tiles = []
            for b in range(4):
                x_t = xpool.tile([128, WP, C], dt_in, tag="x")
                nc.sync.dma_start(out=x_t, in_=xx[img, b * HO : b * HO + 128, :, :])
                x_tiles.append(x_t)

            for b in range(4):
                y_t = ypool.tile([128, W, C], mybir.dt.float32, tag="y")
                for co in range(4):
                    psum_t = pspool.tile([128, 512], mybir.dt.float32, tag="ps")
                    do_co(psum_t, x_tiles[b], co, HO, t_sb)
                    evict(psum_t, y_t, co, HO)
                nc.sync.dma_start(
                    out=yy[img, b * HO : (b + 1) * HO, :, :], in_=y_t[0:HO, :, :]
                )

        if ho_rem:
            x_t = tailpool.tile([128, WP, C], dt_in, tag="xt")
            nc.sync.dma_start(out=x_t, in_=xt_d[:, :, :])
            y_t = tailpool.tile([128, W, C], mybir.dt.float32, tag="yt")
            for co in range(4):
                psum_t = pspool.tile([128, 512], mybir.dt.float32, tag="ps")
                do_co(psum_t, x_t, co, 4 * ho_rem, tt_sb)
                evict(psum_t, y_t, co, 4 * ho_rem)
            for i in range(B_SH):
                nc.sync.dma_start(
                    out=yy[i, 4 * HO : H, :, :],
                    in_=y_t[ho_rem * i : ho_rem * (i + 1), :, :],
                )

    nc.compile()
    return nc


def kernel(**inputs):
    global LAST_RESULTS
    import concourse.mybir as mybir
    from concourse.bass_utils import run_bass_kernel_spmd

    x = np.asarray(inputs["x"], np.float32)
    index = _routing_index(inputs["logits"], inputs["g"])
    w = np.asarray(inputs[f"w{index}"], np.float32)
    gamma = np.asarray(inputs["gamma"], np.float32)[index]
    beta = np.asarray(inputs["beta"], np.float32)[index]
    mean = np.asarray(inputs["mean"], np.float32)[index]
    var = np.asarray(inputs["var"], np.float32)[index]

    inv = (gamma * (1.0 / np.sqrt(var + np.float32(EPS)))).astype(np.float32)
    bvec = (beta - mean * inv).astype(np.float32)

    K = KSIZES[index]
    groups = GROUPS[index]
    pad = K // 2
    HO = 128 - 2 * pad
    ho_rem = H - 4 * HO
    hin_rem = ho_rem + 2 * pad
    cin_g = C // groups
    S = 4 * K * cin_g

    if MODE == "bf16":
        import ml_dtypes

        np_dt = ml_dtypes.bfloat16
        dt_in = mybir.dt.bfloat16
    elif MODE == "f32r":
        np_dt = np.float32
        dt_in = mybir.dt.float32r
    else:
        np_dt = np.float32
        dt_in = mybir.dt.float32

    tfull, ttail, pairs = _build_toeplitz(w, K, groups, HO, ho_rem, np_dt)
    nc = _build_program(K, pairs, S, HO, ho_rem, inv, bvec, dt_in)

    xc = x.astype(np_dt) if np_dt is not np.float32 else x
    WP, HP = W + 2 * pad, H + 2 * pad
    in_maps = []
    for c in range(N_CORES):
        shard = xc[c * B_SH : (c + 1) * B_SH]
        xpad = np.zeros((B_SH, HP, WP, C), np_dt)
        xpad[:, pad : pad + H, pad : pad + W, :] = shard
        m = {"xpad": xpad, "tfull": tfull}
        if ho_rem:
            xtail = np.zeros((128, WP, C), np_dt)
            for i in range(B_SH):
                xtail[32 * i : 32 * i + hin_rem] = xpad[i, 4 * HO : 4 * HO + hin_rem]
            m["xtail"] = xtail
            m["ttail"] = ttail
        in_maps.append(m)

    _ensure_ntff_hook()
    res = run_bass_kernel_spmd(nc, in_maps, core_ids=list(range(N_CORES)))
    LAST_RESULTS = res
    y = np.concatenate([res.results[c]["y"] for c in range(N_CORES)], axis=0)
    return y
